# revision 21
# baseline (speedup 1.0000x reference)
"""Trainium2 Bass kernel: BoundaryInjectedMessagePassingLayer (GNN message passing).

Approach
--------
The per-edge message GEMMs factor through the concat:
    m_int[e] = x[s_e] @ W_iil.T + x[t_e] @ W_iir.T + b_ii
so the scatter-mean decomposes into
  * segment-sums of RAW node features over incoming edges (gather + one-hot
    matmul accumulation on the tensor engine), and
  * count-weighted per-node terms (folded into the final GEMM with
    host-precomputed count ratios).
All per-edge GEMMs disappear; the edge weights are applied ONCE per node after
aggregation (linearity).  Final updates fold into a single fused GEMM per
128-node tile with 8 accumulating matmuls (incl. K=4 bias/count-ratio outer
product and K=1 bias broadcast tricks).

Sharding: nodes are range-sharded across the 8 cores (6250 nodes each), so all
outputs are disjoint and NO collectives are needed.  Gather tables (x, x_bound,
u) are replicated.  Boundary/control row updates are row-sharded.

Host preprocessing is limited to graph-structure work: masks, counting sort of
edges by target, per-tile block layout, per-node degree ratios, and weight
folding (tiny [256,128] matmuls).  All O(E*D)/O(N*D*DH) value compute runs on
device.
"""

import os
import sys
from contextlib import ExitStack

import numpy as np

for _p in ("/opt/trn_rl_repo", "/root/.axon_site/_ro/trn_rl_repo"):
    if os.path.isdir(_p) and _p not in sys.path:
        sys.path.insert(0, _p)
        break

import ml_dtypes  # noqa: E402

import concourse.bass as bass  # noqa: E402
import concourse.tile as tile  # noqa: E402
from concourse import bacc  # noqa: E402
from concourse import mybir  # noqa: E402
from concourse.bass_utils import run_bass_kernel_spmd  # noqa: E402
from concourse.masks import make_identity  # noqa: E402

BF16 = ml_dtypes.bfloat16
STREAMS = ("ilo", "ihi", "b", "c")


class Cfg:
    def __init__(self, N=50000, D=128, DC=64, DH=256, NCORE=8, EB=20000, EC=10000,
                 GC=8):
        self.N, self.D, self.DC, self.DH, self.NCORE = N, D, DC, DH, NCORE
        self.EB, self.EC = EB, EC
        self.P = 128
        assert N % NCORE == 0 and self.D == 128 and DH <= 512
        self.NPC = N // NCORE
        self.TPC = -(-self.NPC // self.P)
        self.NPAD = self.TPC * self.P
        assert EB % NCORE == 0 and EC % NCORE == 0
        self.BPC = EB // NCORE
        self.BT = -(-self.BPC // self.P)
        self.BPAD = self.BT * self.P
        self.CPC = EC // NCORE
        self.CT = -(-self.CPC // self.P)
        self.CPAD = self.CT * self.P
        self.GC = GC
        self.H = (N + 1) // 2 if N > 32768 else N  # int16 gather-table split
        assert self.H <= 32768 and N - self.H <= 32768


def _edge_stream(cfg, tgt, src):
    """Counting-sort edges by target node; lay out per-core [P, NB] index /
    local-target arrays, 128-edge blocks grouped per 128-node tile.

    Program-uniform across cores: per-tile block count = max over cores."""
    P, NPC, TPC, NCORE = cfg.P, cfg.NPC, cfg.TPC, cfg.NCORE
    order = np.argsort(tgt, kind="stable")
    st = np.asarray(tgt, np.int64)[order]
    ss = np.asarray(src, np.int64)[order]

    bounds = np.empty(NCORE * TPC + 1, np.int64)
    for c in range(NCORE):
        bounds[c * TPC:(c + 1) * TPC] = c * NPC + np.arange(TPC) * P
    bounds[-1] = cfg.N
    ptr = np.searchsorted(st, bounds)
    cnt = np.diff(ptr).reshape(NCORE, TPC)
    nb = -(-cnt // P)
    nb_prog = nb.max(axis=0)  # [TPC]
    col_off = np.concatenate([[0], np.cumsum(nb_prog)]).astype(np.int64)
    NB = int(col_off[-1])
    NBa = max(NB, 1)

    idx_arrs, tgtl_arrs = [], []
    for c in range(NCORE):
        # dma_gather index packing: flat edge slot k lives at [k%16, k//16]
        ia = np.zeros((16, NBa * 8), np.int16)
        ta = np.full((P, NBa), -1.0, np.float32)
        e0, e1 = int(ptr[c * TPC]), int(ptr[(c + 1) * TPC] if c + 1 < NCORE else ptr[-1])
        if e1 > e0:
            e = np.arange(e0, e1)
            stc = st[e] - c * NPC
            tloc = stc // P
            k = e - ptr[c * TPC + tloc]
            kk = col_off[tloc] * P + k  # flat padded slot
            ia[kk % 16, kk // 16] = ss[e].astype(np.int16)
            ta[kk % P, kk // P] = (stc - tloc * P).astype(np.float32)
        idx_arrs.append(np.ascontiguousarray(np.tile(ia, (8, 1))))
        tgtl_arrs.append(ta.astype(BF16))
    return tuple(int(v) for v in nb_prog), NBa, idx_arrs, tgtl_arrs


def _preprocess(cfg, inp):
    N, D, DC, DH, NCORE = cfg.N, cfg.D, cfg.DC, cfg.DH, cfg.NCORE
    P, NPC, NPAD = cfg.P, cfg.NPC, cfg.NPAD

    x = np.ascontiguousarray(np.asarray(inp["x_int"], np.float32).reshape(N, D))
    xb = np.ascontiguousarray(np.asarray(inp["x_bound"], np.float32))
    u = np.ascontiguousarray(np.asarray(inp["u"], np.float32))
    ei = np.asarray(inp["edge_index_int"], np.int64)
    eb = np.asarray(inp["edge_index_bound"], np.int64)
    ec = np.asarray(inp["edge_index_ctrl"], np.int64)
    bni = np.asarray(inp["boundary_node_index"], np.int64)
    cni = np.asarray(inp["control_node_index"], np.int64)

    mask_b = np.isin(eb[0], bni)
    mask_c = np.isin(ec[0], cni)

    H = cfg.H
    lo = ei[0] < H
    streams = {
        "ilo": (ei[1][lo], ei[0][lo]),
        "ihi": (ei[1][~lo], ei[0][~lo] - H),
        "b": (eb[1][mask_b], np.nonzero(mask_b)[0]),
        "c": (ec[1][mask_c], np.nonzero(mask_c)[0]),
    }
    meta = {}
    idx_all, tgtl_all = {}, {}
    counts = {}
    for k, (tg, sr) in streams.items():
        nbp, NB, ia, ta = _edge_stream(cfg, tg, sr)
        meta[k] = (nbp, NB)
        idx_all[k], tgtl_all[k] = ia, ta
        counts[k] = np.bincount(tg, minlength=N).astype(np.float32)

    ctot = counts["ilo"] + counts["ihi"] + counts["b"] + counts["c"]
    rtot = 1.0 / np.maximum(ctot, 1.0)
    rint = (counts["ilo"] + counts["ihi"]) * rtot
    rb = counts["b"] * rtot
    rc = counts["c"] * rtot

    # folded weights
    W_ii, W_bi, W_ci = (np.asarray(inp[k], np.float32) for k in ("W_ii", "W_bi", "W_ci"))
    W_im, W_is = np.asarray(inp["W_im"], np.float32), np.asarray(inp["W_is"], np.float32)
    b_ii, b_bi, b_ci = (np.asarray(inp[k], np.float32) for k in ("b_ii", "b_bi", "b_ci"))
    b_im, b_is = np.asarray(inp["b_im"], np.float32), np.asarray(inp["b_is"], np.float32)
    W_bb, b_bb = np.asarray(inp["W_bb"], np.float32), np.asarray(inp["b_bb"], np.float32)
    W_cc, b_cc = np.asarray(inp["W_cc"], np.float32), np.asarray(inp["b_cc"], np.float32)
    W_bm, b_bm = np.asarray(inp["W_bm"], np.float32), np.asarray(inp["b_bm"], np.float32)
    W_bs, b_bs = np.asarray(inp["W_bs"], np.float32), np.asarray(inp["b_bs"], np.float32)
    W_cm, b_cm = np.asarray(inp["W_cm"], np.float32), np.asarray(inp["b_cm"], np.float32)
    W_cs, b_cs = np.asarray(inp["W_cs"], np.float32), np.asarray(inp["b_cs"], np.float32)

    W_iil, W_iir = W_ii[:, :D], W_ii[:, D:]
    W_bil, W_bir = W_bi[:, :D], W_bi[:, D:]
    W_cil, W_cir = W_ci[:, :DC], W_ci[:, DC:]

    wgt = {
        "wisT": W_is.T,                      # [D, DH]
        "f1T": (W_im @ W_iir).T,             # [D, DH]
        "f2T": (W_im @ W_bir).T,
        "f3T": (W_im @ W_cir).T,
        "g1T": (W_im @ W_iil).T,             # [D, DH]
        "g2T": (W_im @ W_bil).T,
        "g3T": (W_im @ W_cil).T,             # [DC, DH]
        "bias4": np.stack([b_is + b_im, W_im @ b_ii, W_im @ b_bi, W_im @ b_ci]),  # [4, DH]
        "wbT": (W_bs + W_bm @ (W_bb[:, :D] + W_bb[:, D:])).T,        # [D, DH]
        "beff": (b_bs + b_bm + W_bm @ b_bb)[None, :],                # [1, DH]
        "wcT": (W_cs + W_cm @ (W_cc[:, :DC] + W_cc[:, DC:])).T,      # [DC, DH]
        "ceff": (b_cs + b_cm + W_cm @ b_cc)[None, :],                # [1, DH]
    }
    wgt = {k: np.ascontiguousarray(v.astype(BF16)) for k, v in wgt.items()}

    def padded_rows(a, rows):
        out = np.zeros((rows, a.shape[1]), np.float32)
        out[:a.shape[0]] = a
        return out

    in_maps = []
    for c in range(NCORE):
        sl = slice(c * NPC, (c + 1) * NPC)

        def bcast(v):
            vp = np.zeros(NPAD, np.float32)
            vp[:NPC] = v[sl]
            return np.ascontiguousarray(
                np.broadcast_to(vp[None, :], (P, NPAD)).astype(BF16))

        scl4 = np.zeros((4, NPAD), np.float32)
        scl4[0, :NPC] = 1.0
        scl4[1, :NPC] = rint[sl]
        scl4[2, :NPC] = rb[sl]
        scl4[3, :NPC] = rc[sl]

        m = {
            "x_lo": x[:H], "x_hi": np.ascontiguousarray(x[H:]) if H < N else x[:1],
            "xb_tab": xb, "u_tab": u,
            "x_sl": padded_rows(x[sl], NPAD),
            "xb_sl": padded_rows(xb[c * cfg.BPC:(c + 1) * cfg.BPC], cfg.BPAD),
            "u_sl": padded_rows(u[c * cfg.CPC:(c + 1) * cfg.CPC], cfg.CPAD),
            "rtot_b": bcast(rtot), "rint_b": bcast(rint),
            "rb_b": bcast(rb), "rc_b": bcast(rc),
            "scl4": np.ascontiguousarray(scl4.astype(BF16)),
        }
        for k in STREAMS:
            m[f"idx_{k}"] = idx_all[k][c]
            m[f"tgtl_{k}"] = tgtl_all[k][c]
        m.update(wgt)
        in_maps.append(m)

    key = (cfg.N, cfg.NCORE) + tuple(meta[k] for k in STREAMS)
    return {"meta": meta, "in_maps": in_maps, "key": key}


def _build(cfg, meta):
    N, D, DC, DH = cfg.N, cfg.D, cfg.DC, cfg.DH
    P, NPAD, TPC = cfg.P, cfg.NPAD, cfg.TPC
    GC = cfg.GC
    f32, bf16, i32 = mybir.dt.float32, mybir.dt.bfloat16, mybir.dt.int32

    H = cfg.H
    nc = bacc.Bacc("TRN2", target_bir_lowering=False, num_devices=cfg.NCORE)
    dp = nc.declare_dram_parameter
    x_lo = dp("x_lo", [H, D], f32, isOutput=False)
    x_hi = dp("x_hi", [max(N - H, 1), D], f32, isOutput=False)
    xb_tab = dp("xb_tab", [cfg.EB, D], f32, isOutput=False)
    u_tab = dp("u_tab", [cfg.EC, DC], f32, isOutput=False)
    x_sl = dp("x_sl", [NPAD, D], f32, isOutput=False)
    xb_sl = dp("xb_sl", [cfg.BPAD, D], f32, isOutput=False)
    u_sl = dp("u_sl", [cfg.CPAD, DC], f32, isOutput=False)
    scale_d = {k: dp(k, [P, NPAD], bf16, isOutput=False)
               for k in ("rtot_b", "rint_b", "rb_b", "rc_b")}
    scl4_d = dp("scl4", [4, NPAD], bf16, isOutput=False)
    i16 = mybir.dt.int16
    idx_d, tgtl_d = {}, {}
    for k in STREAMS:
        NB = meta[k][1]
        idx_d[k] = dp(f"idx_{k}", [128, NB * 8], i16, isOutput=False)
        tgtl_d[k] = dp(f"tgtl_{k}", [P, NB], bf16, isOutput=False)
    wshapes = {
        "wisT": [D, DH], "f1T": [D, DH], "f2T": [D, DH], "f3T": [D, DH],
        "g1T": [D, DH], "g2T": [D, DH], "g3T": [DC, DH], "bias4": [4, DH],
        "wbT": [D, DH], "beff": [1, DH], "wcT": [DC, DH], "ceff": [1, DH],
    }
    w_d = {k: dp(k, s, bf16, isOutput=False) for k, s in wshapes.items()}
    out_i = dp("out_int", [NPAD, DH], f32, isOutput=True)
    out_b = dp("out_b", [cfg.BPAD, DH], f32, isOutput=True)
    out_c = dp("out_c", [cfg.CPAD, DH], f32, isOutput=True)

    with tile.TileContext(nc) as tc, ExitStack() as ctx:
        pconst = ctx.enter_context(tc.tile_pool(name="pconst", bufs=1))
        pmega = ctx.enter_context(tc.tile_pool(name="pmega", bufs=1))

        # ---- constants ----
        ident = pconst.tile([P, P], bf16, tag="ident")
        make_identity(nc, ident[:])
        iota_i = pconst.tile([P, GC * P], i32, tag="iota_i")
        nc.gpsimd.iota(iota_i[:], pattern=[[0, GC], [1, P]], base=0,
                       channel_multiplier=0)
        iota_rep = pconst.tile([P, GC * P], bf16, tag="iota_rep")
        nc.vector.tensor_copy(iota_rep[:], iota_i[:])
        ones1 = pconst.tile([1, P], bf16, tag="ones1")
        nc.vector.memset(ones1[:], 1.0)

        w_sb = {}
        for k, s in wshapes.items():
            w_sb[k] = pconst.tile(s, bf16, name=f"w_{k}", tag=f"w_{k}")
            nc.sync.dma_start(w_sb[k][:], w_d[k][:])
        scl4_sb = pconst.tile([4, NPAD], bf16, tag="scl4")
        nc.sync.dma_start(scl4_sb[:], scl4_d[:])
        idx_sb, tgtl_sb = {}, {}
        for k in STREAMS:
            NB = meta[k][1]
            idx_sb[k] = pconst.tile([128, NB * 8], i16, name=f"idxsb_{k}", tag=f"idxsb_{k}")
            nc.sync.dma_start(idx_sb[k][:], idx_d[k][:])
            tgtl_sb[k] = pconst.tile([P, NB], bf16, name=f"tgtlsb_{k}", tag=f"tgtlsb_{k}")
            nc.sync.dma_start(tgtl_sb[k][:], tgtl_d[k][:])


        # ---- mega tiles ----
        xT = pmega.tile([P, NPAD], bf16, tag="xT")
        xTs = [pmega.tile([P, NPAD], bf16, name=f"xTs{i}", tag=f"xTs{i}") for i in range(3)]
        SxT = pmega.tile([P, NPAD], bf16, tag="SxT")
        SbT = pmega.tile([P, NPAD], bf16, tag="SbT")
        ScT = pmega.tile([DC, NPAD], bf16, tag="ScT")
        xbT = pmega.tile([P, cfg.BPAD], bf16, tag="xbT")
        uT = pmega.tile([DC, cfg.CPAD], bf16, tag="uT")

        # ---- Phase A: transposes ----
        with tc.tile_pool(name="pscale3", bufs=1) as pscale3:
            sc3 = {}
            for k in ("rint_b", "rb_b", "rc_b"):
                sc3[k] = pscale3.tile([P, NPAD], bf16, name=f"sc_{k}", tag=k)
                nc.sync.dma_start(sc3[k][:], scale_d[k][:])
            with tc.tile_pool(name="ptin", bufs=TPC + cfg.BT + cfg.CT) as pin, \
                 tc.tile_pool(name="ptps", bufs=3, space="PSUM") as pps:
                def transpose_stream(src, ntiles, width, megaT):
                    for t in range(ntiles):
                        ti = pin.tile([P, width], bf16, tag="tin")
                        nc.gpsimd.dma_start(ti[:], src[t * P:(t + 1) * P, :])
                        tp = pps.tile([width, P], bf16, tag="tp")
                        nc.tensor.transpose(out=tp[:], in_=ti[:], identity=ident[:])
                        nc.vector.tensor_copy(megaT[:width, t * P:(t + 1) * P], tp[:])
                transpose_stream(x_sl, TPC, D, xT)
                transpose_stream(xb_sl, cfg.BT, D, xbT)
                transpose_stream(u_sl, cfg.CT, DC, uT)

            for i, rk in enumerate(("rint_b", "rb_b", "rc_b")):
                for t in range(TPC):
                    cs = slice(t * P, (t + 1) * P)
                    nc.vector.tensor_tensor(out=xTs[i][:, cs], in0=xT[:, cs],
                                            in1=sc3[rk][:, cs],
                                            op=mybir.AluOpType.mult)

        tc.strict_bb_all_engine_barrier()

        pscaleT = ctx.enter_context(tc.tile_pool(name="pscaleT", bufs=1))
        rtot_sb_tile = pscaleT.tile([P, NPAD], bf16, name="sc_rtot", tag="rtot_b")
        nc.sync.dma_start(rtot_sb_tile[:], scale_d["rtot_b"][:])

        # ---- Phase B: aggregation (dma_gather + one-hot matmul) ----
        def agg_multi(streams, F, megaS, phase_id):
            """streams: list of (key, table_ap) sharing one [F, NPAD] megaS.

            Per 128-node tile, all streams' 128-edge blocks accumulate into one
            PSUM tile via (gathered rows).T @ onehot; on the last block the
            rtot-scaled result lands in megaS (bf16)."""
            rtot_sb = rtot_sb_tile
            per_stream = {}
            merged_by_tile = [[] for _ in range(TPC)]
            for k, table in streams:
                nbp = meta[k][0]
                blkpos = 0
                groups = []  # per stream: list of (g0_blocks, gcols)
                blk2group = []
                for t, nbt in enumerate(nbp):
                    for _ in range(nbt):
                        merged_by_tile[t].append((k, blkpos))
                        blkpos += 1
                nblk = blkpos
                for g0 in range(0, nblk, GC):
                    groups.append((g0, min(GC, nblk - g0)))
                per_stream[k] = dict(table=table, nblk=nblk, groups=groups,
                                     emitted={}, gb={}, ob={})

            with tc.tile_pool(name=f"pg{phase_id}", bufs=3) as pg, \
                 tc.tile_pool(name=f"po{phase_id}", bufs=3) as po, \
                 tc.tile_pool(name=f"pa{phase_id}", bufs=2, space="PSUM") as pa:
                def ensure_group(k, j):
                    ps = per_stream[k]
                    gi = j // GC
                    if gi in ps["emitted"]:
                        return
                    ps["emitted"][gi] = True
                    g0, gcols = ps["groups"][gi]
                    gbf = pg.tile([P, GC * F], f32, name=f"gbf_{k}", tag=f"gbf_{k}")
                    nc.gpsimd.dma_gather(
                        out_ap=gbf[:, :gcols * F].rearrange("p (g f) -> p g f", g=gcols),
                        in_ap=ps["table"][:],
                        idxs_ap=idx_sb[k][:, g0 * 8:(g0 + gcols) * 8],
                        num_idxs=gcols * P,
                        num_idxs_reg=gcols * P,
                        elem_size=F,
                    )
                    gb = pg.tile([P, GC * F], bf16, name=f"gb_{k}", tag=f"gb_{k}")
                    nc.scalar.copy(gb[:, :gcols * F], gbf[:, :gcols * F])
                    ob = po.tile([P, GC * P], bf16, name=f"ob_{k}", tag=f"ob_{k}")
                    nc.vector.tensor_tensor(
                        out=ob[:, :gcols * P],
                        in0=tgtl_sb[k][:, g0:g0 + gcols].to_broadcast([P, gcols, P]),
                        in1=iota_rep[:, :gcols * P].rearrange("p (g w) -> p g w", g=gcols),
                        op=mybir.AluOpType.is_equal,
                    )
                    ps["gb"][gi], ps["ob"][gi] = gb, ob

                for t in range(TPC):
                    blocks = merged_by_tile[t]
                    if not blocks:
                        nc.vector.memset(megaS[:F, t * P:(t + 1) * P], 0.0)
                        continue
                    acc = pa.tile([F, P], f32, name="acc", tag="acc")
                    for bi, (k, j) in enumerate(blocks):
                        ensure_group(k, j)
                        ps = per_stream[k]
                        gi, jj = j // GC, j % GC
                        nc.tensor.matmul(
                            out=acc[:],
                            lhsT=ps["gb"][gi][:, jj * F:(jj + 1) * F],
                            rhs=ps["ob"][gi][:, jj * P:(jj + 1) * P],
                            start=(bi == 0), stop=(bi == len(blocks) - 1),
                        )
                    nc.vector.tensor_tensor(
                        out=megaS[:F, t * P:(t + 1) * P],
                        in0=acc[:],
                        in1=rtot_sb[:F, t * P:(t + 1) * P],
                        op=mybir.AluOpType.mult,
                    )

        agg_multi([("ilo", x_lo), ("ihi", x_hi)], D, SxT, "x")
        agg_multi([("b", xb_tab)], D, SbT, "b")
        agg_multi([("c", u_tab)], DC, ScT, "c")

        # ---- Phase C: fused final GEMMs ----
        with tc.tile_pool(name="pfin", bufs=2, space="PSUM") as pf, \
             tc.tile_pool(name="pstg", bufs=3) as pstg:
            def mm(acc, lhsT, rhs, start, stop):
                nc.tensor.matmul(out=acc[:], lhsT=lhsT, rhs=rhs[:], start=start, stop=stop)

            for t in range(TPC):
                cs = slice(t * P, (t + 1) * P)
                acc = pf.tile([P, DH], f32, tag="fin")
                mm(acc, xT[:, cs], w_sb["wisT"], True, False)
                mm(acc, xTs[0][:, cs], w_sb["f1T"], False, False)
                mm(acc, xTs[1][:, cs], w_sb["f2T"], False, False)
                mm(acc, xTs[2][:, cs], w_sb["f3T"], False, False)
                mm(acc, SxT[:, cs], w_sb["g1T"], False, False)
                mm(acc, SbT[:, cs], w_sb["g2T"], False, False)
                mm(acc, ScT[:DC, cs], w_sb["g3T"], False, False)
                mm(acc, scl4_sb[:, cs], w_sb["bias4"], False, True)
                st = pstg.tile([P, DH], f32, tag="st")
                nc.vector.tensor_copy(st[:], acc[:])
                nc.sync.dma_start(out_i[t * P:(t + 1) * P, :], st[:])

            for t in range(cfg.BT):
                cs = slice(t * P, (t + 1) * P)
                acc = pf.tile([P, DH], f32, tag="fin")
                mm(acc, xbT[:, cs], w_sb["wbT"], True, False)
                mm(acc, ones1[:, :], w_sb["beff"], False, True)
                st = pstg.tile([P, DH], f32, tag="st")
                nc.vector.tensor_copy(st[:], acc[:])
                nc.sync.dma_start(out_b[t * P:(t + 1) * P, :], st[:])

            for t in range(cfg.CT):
                cs = slice(t * P, (t + 1) * P)
                acc = pf.tile([P, DH], f32, tag="fin")
                mm(acc, uT[:DC, cs], w_sb["wcT"], True, False)
                mm(acc, ones1[:, :], w_sb["ceff"], False, True)
                st = pstg.tile([P, DH], f32, tag="st")
                nc.vector.tensor_copy(st[:], acc[:])
                nc.sync.dma_start(out_c[t * P:(t + 1) * P, :], st[:])

    nc.finalize()
    return nc


_CACHE = {}


LAST_RESULT = None


class _Runner:
    """Cached PJRT execution of a finalized Bacc program across n cores.

    Mirrors bass2jax.run_bass_via_pjrt's multi-core branch, but keeps the
    jitted callable (no recompiles across calls) and exposes a path with
    pre-committed device inputs for benchmarking."""

    def __init__(self, nc, n_cores):
        import jax
        from jax.sharding import Mesh, PartitionSpec, NamedSharding
        from jax.experimental.shard_map import shard_map
        from concourse import bass2jax
        bass2jax.install_neuronx_cc_hook()
        self.jax = jax
        self.n_cores = n_cores
        partition_name = (nc.partition_id_tensor.name
                          if nc.partition_id_tensor else None)
        in_names, out_names, out_avals = [], [], []
        for alloc in nc.m.functions[0].allocations:
            if not isinstance(alloc, mybir.MemoryLocationSet):
                continue
            name = alloc.memorylocations[0].name
            if alloc.kind == "ExternalInput":
                if name != partition_name:
                    in_names.append(name)
            elif alloc.kind == "ExternalOutput":
                out_names.append(name)
                out_avals.append(jax.core.ShapedArray(
                    tuple(alloc.tensor_shape), mybir.dt.np(alloc.dtype)))
        self.in_params = list(in_names)
        self.out_names = out_names
        self.out_avals = out_avals
        all_names = list(in_names) + list(out_names)
        if partition_name is not None:
            all_names.append(partition_name)

        def _body(*args):
            operands = list(args)
            if partition_name is not None:
                operands.append(bass2jax.partition_id_tensor())
            outs = bass2jax._bass_exec_p.bind(
                *operands,
                out_avals=tuple(out_avals),
                in_names=tuple(all_names),
                out_names=tuple(out_names),
                lowering_input_output_aliases=(),
                sim_require_finite=True,
                sim_require_nnan=True,
                nc=nc,
            )
            return tuple(outs)

        devices = jax.devices()[:n_cores]
        self.mesh = Mesh(np.asarray(devices), ("core",))
        nin = len(self.in_params) + len(out_names)
        self.sharding = NamedSharding(self.mesh, PartitionSpec("core"))
        self.fn = jax.jit(shard_map(
            _body, mesh=self.mesh,
            in_specs=(PartitionSpec("core"),) * nin,
            out_specs=(PartitionSpec("core"),) * len(out_names),
            check_rep=False), keep_unused=True)
        self.zeros = None

    def put_inputs(self, in_maps):
        """Concatenate per-core inputs and commit to devices."""
        concat = [np.concatenate([np.asarray(m[k]) for m in in_maps], axis=0)
                  for k in self.in_params]
        args = [self.jax.device_put(a, self.sharding) for a in concat]
        if self.zeros is None:
            self.zeros = [
                self.jax.device_put(
                    np.zeros((self.n_cores * av.shape[0],) + av.shape[1:],
                             av.dtype), self.sharding)
                for av in self.out_avals]
        return args

    def run(self, args):
        return self.fn(*args, *self.zeros)

    def run_to_numpy(self, args):
        outs = self.run(args)
        res = []
        for c in range(self.n_cores):
            res.append({
                name: np.asarray(outs[i]).reshape(
                    (self.n_cores,) + self.out_avals[i].shape)[c]
                for i, name in enumerate(self.out_names)})
        return res


def kernel(**inputs):
    global LAST_RESULT
    cfg = Cfg()
    prep = _preprocess(cfg, inputs)
    key = prep["key"]
    ent = _CACHE.get(key)
    if ent is None:
        nc = _build(cfg, prep["meta"])
        ent = {"nc": nc, "runner": _Runner(nc, cfg.NCORE)}
        _CACHE[key] = ent
    runner = ent["runner"]
    args = runner.put_inputs(prep["in_maps"])
    results = runner.run_to_numpy(args)
    ent["last_args"] = args
    LAST_RESULT = ent
    interior = np.concatenate(
        [results[c]["out_int"][:cfg.NPC] for c in range(cfg.NCORE)], axis=0)
    boundary = np.concatenate(
        [results[c]["out_b"][:cfg.BPC] for c in range(cfg.NCORE)], axis=0)
    control = np.concatenate(
        [results[c]["out_c"][:cfg.CPC] for c in range(cfg.NCORE)], axis=0)
    return (np.ascontiguousarray(interior, dtype=np.float32),
            np.ascontiguousarray(boundary, dtype=np.float32),
            np.ascontiguousarray(control, dtype=np.float32))


def _tiny_runner(ncore=8):
    """Trivial 8-core NEFF (one small DMA in/out) to calibrate dispatch floor."""
    f32 = mybir.dt.float32
    nc = bacc.Bacc("TRN2", target_bir_lowering=False, num_devices=ncore)
    a = nc.declare_dram_parameter("a", [128, 256], f32, isOutput=False)
    o = nc.declare_dram_parameter("o", [128, 256], f32, isOutput=True)
    with tile.TileContext(nc) as tc, ExitStack() as ctx:
        pool = ctx.enter_context(tc.tile_pool(name="pool", bufs=1))
        t = pool.tile([128, 256], f32)
        nc.sync.dma_start(t[:], a[:])
        nc.sync.dma_start(o[:], t[:])
    nc.finalize()
    r = _Runner(nc, ncore)
    args = r.put_inputs([{"a": np.zeros((128, 256), np.float32)}] * ncore)
    return r, args


# revision 22
# speedup vs baseline: 1.0037x; 1.0037x over previous
"""Trainium2 Bass kernel: BoundaryInjectedMessagePassingLayer (GNN message passing).

Approach
--------
The per-edge message GEMMs factor through the concat:
    m_int[e] = x[s_e] @ W_iil.T + x[t_e] @ W_iir.T + b_ii
so the scatter-mean decomposes into
  * segment-sums of RAW node features over incoming edges (gather + one-hot
    matmul accumulation on the tensor engine), and
  * count-weighted per-node terms (folded into the final GEMM with
    host-precomputed count ratios).
All per-edge GEMMs disappear; the edge weights are applied ONCE per node after
aggregation (linearity).  Final updates fold into a single fused GEMM per
128-node tile with 8 accumulating matmuls (incl. K=4 bias/count-ratio outer
product and K=1 bias broadcast tricks).

Sharding: nodes are range-sharded across the 8 cores (6250 nodes each), so all
outputs are disjoint and NO collectives are needed.  Gather tables (x, x_bound,
u) are replicated.  Boundary/control row updates are row-sharded.

Host preprocessing is limited to graph-structure work: masks, counting sort of
edges by target, per-tile block layout, per-node degree ratios, and weight
folding (tiny [256,128] matmuls).  All O(E*D)/O(N*D*DH) value compute runs on
device.
"""

import os
import sys
from contextlib import ExitStack

import numpy as np

for _p in ("/opt/trn_rl_repo", "/root/.axon_site/_ro/trn_rl_repo"):
    if os.path.isdir(_p) and _p not in sys.path:
        sys.path.insert(0, _p)
        break

import ml_dtypes  # noqa: E402

import concourse.bass as bass  # noqa: E402
import concourse.tile as tile  # noqa: E402
from concourse import bacc  # noqa: E402
from concourse import mybir  # noqa: E402
from concourse.bass_utils import run_bass_kernel_spmd  # noqa: E402
from concourse.masks import make_identity  # noqa: E402

BF16 = ml_dtypes.bfloat16
STREAMS = ("ilo", "ihi", "b", "c")


class Cfg:
    def __init__(self, N=50000, D=128, DC=64, DH=256, NCORE=8, EB=20000, EC=10000,
                 GC=8):
        self.N, self.D, self.DC, self.DH, self.NCORE = N, D, DC, DH, NCORE
        self.EB, self.EC = EB, EC
        self.P = 128
        assert N % NCORE == 0 and self.D == 128 and DH <= 512
        self.NPC = N // NCORE
        self.TPC = -(-self.NPC // self.P)
        self.NPAD = self.TPC * self.P
        assert EB % NCORE == 0 and EC % NCORE == 0
        self.BPC = EB // NCORE
        self.BT = -(-self.BPC // self.P)
        self.BPAD = self.BT * self.P
        self.CPC = EC // NCORE
        self.CT = -(-self.CPC // self.P)
        self.CPAD = self.CT * self.P
        self.GC = GC
        self.H = (N + 1) // 2 if N > 32768 else N  # int16 gather-table split
        assert self.H <= 32768 and N - self.H <= 32768


def _edge_stream(cfg, tgt, src):
    """Counting-sort edges by target node; lay out per-core [P, NB] index /
    local-target arrays, 128-edge blocks grouped per 128-node tile.

    Program-uniform across cores: per-tile block count = max over cores."""
    P, NPC, TPC, NCORE = cfg.P, cfg.NPC, cfg.TPC, cfg.NCORE
    order = np.argsort(tgt, kind="stable")
    st = np.asarray(tgt, np.int64)[order]
    ss = np.asarray(src, np.int64)[order]

    bounds = np.empty(NCORE * TPC + 1, np.int64)
    for c in range(NCORE):
        bounds[c * TPC:(c + 1) * TPC] = c * NPC + np.arange(TPC) * P
    bounds[-1] = cfg.N
    ptr = np.searchsorted(st, bounds)
    cnt = np.diff(ptr).reshape(NCORE, TPC)
    nb = -(-cnt // P)
    nb_prog = nb.max(axis=0)  # [TPC]
    col_off = np.concatenate([[0], np.cumsum(nb_prog)]).astype(np.int64)
    NB = int(col_off[-1])
    NBa = max(NB, 1)

    idx_arrs, tgtl_arrs = [], []
    for c in range(NCORE):
        # dma_gather index packing: flat edge slot k lives at [k%16, k//16]
        ia = np.zeros((16, NBa * 8), np.int16)
        ta = np.full((P, NBa), -1.0, np.float32)
        e0, e1 = int(ptr[c * TPC]), int(ptr[(c + 1) * TPC] if c + 1 < NCORE else ptr[-1])
        if e1 > e0:
            e = np.arange(e0, e1)
            stc = st[e] - c * NPC
            tloc = stc // P
            k = e - ptr[c * TPC + tloc]
            kk = col_off[tloc] * P + k  # flat padded slot
            ia[kk % 16, kk // 16] = ss[e].astype(np.int16)
            ta[kk % P, kk // P] = (stc - tloc * P).astype(np.float32)
        idx_arrs.append(np.ascontiguousarray(np.tile(ia, (8, 1))))
        tgtl_arrs.append(ta.astype(BF16))
    return tuple(int(v) for v in nb_prog), NBa, idx_arrs, tgtl_arrs


def _preprocess(cfg, inp):
    N, D, DC, DH, NCORE = cfg.N, cfg.D, cfg.DC, cfg.DH, cfg.NCORE
    P, NPC, NPAD = cfg.P, cfg.NPC, cfg.NPAD

    x = np.ascontiguousarray(np.asarray(inp["x_int"], np.float32).reshape(N, D))
    xb = np.ascontiguousarray(np.asarray(inp["x_bound"], np.float32))
    u = np.ascontiguousarray(np.asarray(inp["u"], np.float32))
    ei = np.asarray(inp["edge_index_int"], np.int64)
    eb = np.asarray(inp["edge_index_bound"], np.int64)
    ec = np.asarray(inp["edge_index_ctrl"], np.int64)
    bni = np.asarray(inp["boundary_node_index"], np.int64)
    cni = np.asarray(inp["control_node_index"], np.int64)

    mask_b = np.isin(eb[0], bni)
    mask_c = np.isin(ec[0], cni)

    H = cfg.H
    lo = ei[0] < H
    streams = {
        "ilo": (ei[1][lo], ei[0][lo]),
        "ihi": (ei[1][~lo], ei[0][~lo] - H),
        "b": (eb[1][mask_b], np.nonzero(mask_b)[0]),
        "c": (ec[1][mask_c], np.nonzero(mask_c)[0]),
    }
    meta = {}
    idx_all, tgtl_all = {}, {}
    counts = {}
    for k, (tg, sr) in streams.items():
        nbp, NB, ia, ta = _edge_stream(cfg, tg, sr)
        meta[k] = (nbp, NB)
        idx_all[k], tgtl_all[k] = ia, ta
        counts[k] = np.bincount(tg, minlength=N).astype(np.float32)

    ctot = counts["ilo"] + counts["ihi"] + counts["b"] + counts["c"]
    rtot = 1.0 / np.maximum(ctot, 1.0)
    rint = (counts["ilo"] + counts["ihi"]) * rtot
    rb = counts["b"] * rtot
    rc = counts["c"] * rtot

    # folded weights
    W_ii, W_bi, W_ci = (np.asarray(inp[k], np.float32) for k in ("W_ii", "W_bi", "W_ci"))
    W_im, W_is = np.asarray(inp["W_im"], np.float32), np.asarray(inp["W_is"], np.float32)
    b_ii, b_bi, b_ci = (np.asarray(inp[k], np.float32) for k in ("b_ii", "b_bi", "b_ci"))
    b_im, b_is = np.asarray(inp["b_im"], np.float32), np.asarray(inp["b_is"], np.float32)
    W_bb, b_bb = np.asarray(inp["W_bb"], np.float32), np.asarray(inp["b_bb"], np.float32)
    W_cc, b_cc = np.asarray(inp["W_cc"], np.float32), np.asarray(inp["b_cc"], np.float32)
    W_bm, b_bm = np.asarray(inp["W_bm"], np.float32), np.asarray(inp["b_bm"], np.float32)
    W_bs, b_bs = np.asarray(inp["W_bs"], np.float32), np.asarray(inp["b_bs"], np.float32)
    W_cm, b_cm = np.asarray(inp["W_cm"], np.float32), np.asarray(inp["b_cm"], np.float32)
    W_cs, b_cs = np.asarray(inp["W_cs"], np.float32), np.asarray(inp["b_cs"], np.float32)

    W_iil, W_iir = W_ii[:, :D], W_ii[:, D:]
    W_bil, W_bir = W_bi[:, :D], W_bi[:, D:]
    W_cil, W_cir = W_ci[:, :DC], W_ci[:, DC:]

    wgt = {
        "wisT": W_is.T,                      # [D, DH]
        "f1T": (W_im @ W_iir).T,             # [D, DH]
        "f2T": (W_im @ W_bir).T,
        "f3T": (W_im @ W_cir).T,
        "g1T": (W_im @ W_iil).T,             # [D, DH]
        "g2T": (W_im @ W_bil).T,
        "g3T": (W_im @ W_cil).T,             # [DC, DH]
        "bias4": np.stack([b_is + b_im, W_im @ b_ii, W_im @ b_bi, W_im @ b_ci]),  # [4, DH]
        "wbT": (W_bs + W_bm @ (W_bb[:, :D] + W_bb[:, D:])).T,        # [D, DH]
        "beff": (b_bs + b_bm + W_bm @ b_bb)[None, :],                # [1, DH]
        "wcT": (W_cs + W_cm @ (W_cc[:, :DC] + W_cc[:, DC:])).T,      # [DC, DH]
        "ceff": (b_cs + b_cm + W_cm @ b_cc)[None, :],                # [1, DH]
    }
    wgt = {k: np.ascontiguousarray(v.astype(BF16)) for k, v in wgt.items()}

    def padded_rows(a, rows):
        out = np.zeros((rows, a.shape[1]), np.float32)
        out[:a.shape[0]] = a
        return out

    in_maps = []
    for c in range(NCORE):
        sl = slice(c * NPC, (c + 1) * NPC)

        def bcast(v):
            vp = np.zeros(NPAD, np.float32)
            vp[:NPC] = v[sl]
            return np.ascontiguousarray(
                np.broadcast_to(vp[None, :], (P, NPAD)).astype(BF16))

        scl4 = np.zeros((4, NPAD), np.float32)
        scl4[0, :NPC] = 1.0
        scl4[1, :NPC] = rint[sl]
        scl4[2, :NPC] = rb[sl]
        scl4[3, :NPC] = rc[sl]

        m = {
            "x_lo": x[:H], "x_hi": np.ascontiguousarray(x[H:]) if H < N else x[:1],
            "xb_tab": xb, "u_tab": u,
            "x_sl": padded_rows(x[sl], NPAD),
            "xb_sl": padded_rows(xb[c * cfg.BPC:(c + 1) * cfg.BPC], cfg.BPAD),
            "u_sl": padded_rows(u[c * cfg.CPC:(c + 1) * cfg.CPC], cfg.CPAD),
            "rtot_b": bcast(rtot), "rint_b": bcast(rint),
            "rb_b": bcast(rb), "rc_b": bcast(rc),
            "scl4": np.ascontiguousarray(scl4.astype(BF16)),
        }
        for k in STREAMS:
            m[f"idx_{k}"] = idx_all[k][c]
            m[f"tgtl_{k}"] = tgtl_all[k][c]
        m.update(wgt)
        in_maps.append(m)

    key = (cfg.N, cfg.NCORE) + tuple(meta[k] for k in STREAMS)
    return {"meta": meta, "in_maps": in_maps, "key": key}


def _build(cfg, meta):
    N, D, DC, DH = cfg.N, cfg.D, cfg.DC, cfg.DH
    P, NPAD, TPC = cfg.P, cfg.NPAD, cfg.TPC
    GC = cfg.GC
    f32, bf16, i32 = mybir.dt.float32, mybir.dt.bfloat16, mybir.dt.int32

    H = cfg.H
    nc = bacc.Bacc("TRN2", target_bir_lowering=False, num_devices=cfg.NCORE)
    dp = nc.declare_dram_parameter
    x_lo = dp("x_lo", [H, D], f32, isOutput=False)
    x_hi = dp("x_hi", [max(N - H, 1), D], f32, isOutput=False)
    xb_tab = dp("xb_tab", [cfg.EB, D], f32, isOutput=False)
    u_tab = dp("u_tab", [cfg.EC, DC], f32, isOutput=False)
    x_sl = dp("x_sl", [NPAD, D], f32, isOutput=False)
    xb_sl = dp("xb_sl", [cfg.BPAD, D], f32, isOutput=False)
    u_sl = dp("u_sl", [cfg.CPAD, DC], f32, isOutput=False)
    scale_d = {k: dp(k, [P, NPAD], bf16, isOutput=False)
               for k in ("rtot_b", "rint_b", "rb_b", "rc_b")}
    scl4_d = dp("scl4", [4, NPAD], bf16, isOutput=False)
    i16 = mybir.dt.int16
    idx_d, tgtl_d = {}, {}
    for k in STREAMS:
        NB = meta[k][1]
        idx_d[k] = dp(f"idx_{k}", [128, NB * 8], i16, isOutput=False)
        tgtl_d[k] = dp(f"tgtl_{k}", [P, NB], bf16, isOutput=False)
    wshapes = {
        "wisT": [D, DH], "f1T": [D, DH], "f2T": [D, DH], "f3T": [D, DH],
        "g1T": [D, DH], "g2T": [D, DH], "g3T": [DC, DH], "bias4": [4, DH],
        "wbT": [D, DH], "beff": [1, DH], "wcT": [DC, DH], "ceff": [1, DH],
    }
    w_d = {k: dp(k, s, bf16, isOutput=False) for k, s in wshapes.items()}
    out_i = dp("out_int", [NPAD, DH], f32, isOutput=True)
    out_b = dp("out_b", [cfg.BPAD, DH], f32, isOutput=True)
    out_c = dp("out_c", [cfg.CPAD, DH], f32, isOutput=True)

    with tile.TileContext(nc) as tc, ExitStack() as ctx:
        pconst = ctx.enter_context(tc.tile_pool(name="pconst", bufs=1))
        pmega = ctx.enter_context(tc.tile_pool(name="pmega", bufs=1))

        # ---- constants ----
        ident = pconst.tile([P, P], bf16, tag="ident")
        make_identity(nc, ident[:])
        iota_i = pconst.tile([P, GC * P], i32, tag="iota_i")
        nc.gpsimd.iota(iota_i[:], pattern=[[0, GC], [1, P]], base=0,
                       channel_multiplier=0)
        iota_rep = pconst.tile([P, GC * P], bf16, tag="iota_rep")
        nc.vector.tensor_copy(iota_rep[:], iota_i[:])
        ones1 = pconst.tile([1, P], bf16, tag="ones1")
        nc.vector.memset(ones1[:], 1.0)

        w_sb = {}
        for k, s in wshapes.items():
            w_sb[k] = pconst.tile(s, bf16, name=f"w_{k}", tag=f"w_{k}")
            nc.sync.dma_start(w_sb[k][:], w_d[k][:])
        scl4_sb = pconst.tile([4, NPAD], bf16, tag="scl4")
        nc.sync.dma_start(scl4_sb[:], scl4_d[:])
        idx_sb, tgtl_sb = {}, {}
        for k in STREAMS:
            NB = meta[k][1]
            idx_sb[k] = pconst.tile([128, NB * 8], i16, name=f"idxsb_{k}", tag=f"idxsb_{k}")
            nc.sync.dma_start(idx_sb[k][:], idx_d[k][:])
            tgtl_sb[k] = pconst.tile([P, NB], bf16, name=f"tgtlsb_{k}", tag=f"tgtlsb_{k}")
            nc.sync.dma_start(tgtl_sb[k][:], tgtl_d[k][:])


        # ---- mega tiles ----
        xT = pmega.tile([P, NPAD], bf16, tag="xT")
        xTs = [pmega.tile([P, NPAD], bf16, name=f"xTs{i}", tag=f"xTs{i}") for i in range(3)]
        SxT = pmega.tile([P, NPAD], bf16, tag="SxT")
        SbT = pmega.tile([P, NPAD], bf16, tag="SbT")
        ScT = pmega.tile([DC, NPAD], bf16, tag="ScT")
        xbT = pmega.tile([P, cfg.BPAD], bf16, tag="xbT")
        uT = pmega.tile([DC, cfg.CPAD], bf16, tag="uT")

        # ---- Phase A: transposes ----
        with tc.tile_pool(name="pscale3", bufs=1) as pscale3:
            sc3 = {}
            for k in ("rint_b", "rb_b", "rc_b"):
                sc3[k] = pscale3.tile([P, NPAD], bf16, name=f"sc_{k}", tag=k)
                nc.sync.dma_start(sc3[k][:], scale_d[k][:])
            with tc.tile_pool(name="ptin", bufs=TPC + cfg.BT + cfg.CT) as pin, \
                 tc.tile_pool(name="ptps", bufs=3, space="PSUM") as pps:
                def transpose_stream(src, ntiles, width, megaT):
                    for t in range(ntiles):
                        ti = pin.tile([P, width], bf16, tag="tin")
                        nc.gpsimd.dma_start(ti[:], src[t * P:(t + 1) * P, :])
                        tp = pps.tile([width, P], bf16, tag="tp")
                        nc.tensor.transpose(out=tp[:], in_=ti[:], identity=ident[:])
                        nc.vector.tensor_copy(megaT[:width, t * P:(t + 1) * P], tp[:])
                transpose_stream(x_sl, TPC, D, xT)
                transpose_stream(xb_sl, cfg.BT, D, xbT)
                transpose_stream(u_sl, cfg.CT, DC, uT)

            for i, rk in enumerate(("rint_b", "rb_b", "rc_b")):
                for t in range(TPC):
                    cs = slice(t * P, (t + 1) * P)
                    nc.vector.tensor_tensor(out=xTs[i][:, cs], in0=xT[:, cs],
                                            in1=sc3[rk][:, cs],
                                            op=mybir.AluOpType.mult)

        tc.strict_bb_all_engine_barrier()

        pscaleT = ctx.enter_context(tc.tile_pool(name="pscaleT", bufs=1))
        rtot_sb_tile = pscaleT.tile([P, NPAD], bf16, name="sc_rtot", tag="rtot_b")
        nc.sync.dma_start(rtot_sb_tile[:], scale_d["rtot_b"][:])

        # ---- Phase B: aggregation (dma_gather + one-hot matmul) ----
        def agg_multi(streams, F, megaS, phase_id):
            """streams: list of (key, table_ap) sharing one [F, NPAD] megaS.

            Per 128-node tile, all streams' 128-edge blocks accumulate into one
            PSUM tile via (gathered rows).T @ onehot; on the last block the
            rtot-scaled result lands in megaS (bf16)."""
            rtot_sb = rtot_sb_tile
            per_stream = {}
            merged_by_tile = [[] for _ in range(TPC)]
            for k, table in streams:
                nbp = meta[k][0]
                blkpos = 0
                groups = []  # per stream: list of (g0_blocks, gcols)
                blk2group = []
                for t, nbt in enumerate(nbp):
                    for _ in range(nbt):
                        merged_by_tile[t].append((k, blkpos))
                        blkpos += 1
                nblk = blkpos
                for g0 in range(0, nblk, GC):
                    groups.append((g0, min(GC, nblk - g0)))
                per_stream[k] = dict(table=table, nblk=nblk, groups=groups,
                                     emitted={}, gb={}, ob={})

            with tc.tile_pool(name=f"pg{phase_id}", bufs=3) as pg, \
                 tc.tile_pool(name=f"po{phase_id}", bufs=3) as po, \
                 tc.tile_pool(name=f"pa{phase_id}", bufs=2, space="PSUM") as pa:
                def ensure_group(k, j):
                    ps = per_stream[k]
                    gi = j // GC
                    if gi in ps["emitted"]:
                        return
                    ps["emitted"][gi] = True
                    g0, gcols = ps["groups"][gi]
                    gbf = pg.tile([P, GC * F], f32, name=f"gbf_{k}", tag=f"gbf_{k}")
                    nc.gpsimd.dma_gather(
                        out_ap=gbf[:, :gcols * F].rearrange("p (g f) -> p g f", g=gcols),
                        in_ap=ps["table"][:],
                        idxs_ap=idx_sb[k][:, g0 * 8:(g0 + gcols) * 8],
                        num_idxs=gcols * P,
                        num_idxs_reg=gcols * P,
                        elem_size=F,
                    )
                    gb = pg.tile([P, GC * F], bf16, name=f"gb_{k}", tag=f"gb_{k}")
                    nc.scalar.copy(gb[:, :gcols * F], gbf[:, :gcols * F])
                    ob = po.tile([P, GC * P], bf16, name=f"ob_{k}", tag=f"ob_{k}")
                    nc.vector.tensor_tensor(
                        out=ob[:, :gcols * P],
                        in0=tgtl_sb[k][:, g0:g0 + gcols].to_broadcast([P, gcols, P]),
                        in1=iota_rep[:, :gcols * P].rearrange("p (g w) -> p g w", g=gcols),
                        op=mybir.AluOpType.is_equal,
                    )
                    ps["gb"][gi], ps["ob"][gi] = gb, ob

                for t in range(TPC):
                    blocks = merged_by_tile[t]
                    if not blocks:
                        nc.vector.memset(megaS[:F, t * P:(t + 1) * P], 0.0)
                        continue
                    acc = pa.tile([F, P], f32, name="acc", tag="acc")
                    for bi, (k, j) in enumerate(blocks):
                        ensure_group(k, j)
                        ps = per_stream[k]
                        gi, jj = j // GC, j % GC
                        nc.tensor.matmul(
                            out=acc[:],
                            lhsT=ps["gb"][gi][:, jj * F:(jj + 1) * F],
                            rhs=ps["ob"][gi][:, jj * P:(jj + 1) * P],
                            start=(bi == 0), stop=(bi == len(blocks) - 1),
                        )
                    nc.vector.tensor_tensor(
                        out=megaS[:F, t * P:(t + 1) * P],
                        in0=acc[:],
                        in1=rtot_sb[:F, t * P:(t + 1) * P],
                        op=mybir.AluOpType.mult,
                    )

        agg_multi([("ilo", x_lo), ("ihi", x_hi)], D, SxT, "x")
        agg_multi([("b", xb_tab)], D, SbT, "b")
        agg_multi([("c", u_tab)], DC, ScT, "c")

        # ---- Phase C: fused final GEMMs ----
        with tc.tile_pool(name="pfin", bufs=2, space="PSUM") as pf, \
             tc.tile_pool(name="pstg", bufs=3) as pstg:
            def mm(acc, lhsT, rhs, start, stop):
                nc.tensor.matmul(out=acc[:], lhsT=lhsT, rhs=rhs[:], start=start, stop=stop)

            for t in range(TPC):
                cs = slice(t * P, (t + 1) * P)
                acc = pf.tile([P, DH], f32, tag="fin")
                mm(acc, xT[:, cs], w_sb["wisT"], True, False)
                mm(acc, xTs[0][:, cs], w_sb["f1T"], False, False)
                mm(acc, xTs[1][:, cs], w_sb["f2T"], False, False)
                mm(acc, xTs[2][:, cs], w_sb["f3T"], False, False)
                mm(acc, SxT[:, cs], w_sb["g1T"], False, False)
                mm(acc, SbT[:, cs], w_sb["g2T"], False, False)
                mm(acc, ScT[:DC, cs], w_sb["g3T"], False, False)
                mm(acc, scl4_sb[:, cs], w_sb["bias4"], False, True)
                st = pstg.tile([P, DH], f32, tag="st")
                nc.vector.tensor_copy(st[:], acc[:])
                nc.sync.dma_start(out_i[t * P:(t + 1) * P, :], st[:])

            for t in range(cfg.BT):
                cs = slice(t * P, (t + 1) * P)
                acc = pf.tile([P, DH], f32, tag="fin")
                mm(acc, xbT[:, cs], w_sb["wbT"], True, False)
                mm(acc, ones1[:, :], w_sb["beff"], False, True)
                st = pstg.tile([P, DH], f32, tag="st")
                nc.vector.tensor_copy(st[:], acc[:])
                nc.sync.dma_start(out_b[t * P:(t + 1) * P, :], st[:])

            for t in range(cfg.CT):
                cs = slice(t * P, (t + 1) * P)
                acc = pf.tile([P, DH], f32, tag="fin")
                mm(acc, uT[:DC, cs], w_sb["wcT"], True, False)
                mm(acc, ones1[:, :], w_sb["ceff"], False, True)
                st = pstg.tile([P, DH], f32, tag="st")
                nc.vector.tensor_copy(st[:], acc[:])
                nc.sync.dma_start(out_c[t * P:(t + 1) * P, :], st[:])

    nc.finalize()
    return nc


_CACHE = {}


LAST_RESULT = None


class _Runner:
    """Cached PJRT execution of a finalized Bacc program across n cores.

    Mirrors bass2jax.run_bass_via_pjrt's multi-core branch, but keeps the
    jitted callable (no recompiles across calls) and exposes a path with
    pre-committed device inputs for benchmarking."""

    def __init__(self, nc, n_cores):
        import jax
        from jax.sharding import Mesh, PartitionSpec, NamedSharding
        from jax.experimental.shard_map import shard_map
        from concourse import bass2jax
        bass2jax.install_neuronx_cc_hook()
        self.jax = jax
        self.n_cores = n_cores
        partition_name = (nc.partition_id_tensor.name
                          if nc.partition_id_tensor else None)
        in_names, out_names, out_avals = [], [], []
        for alloc in nc.m.functions[0].allocations:
            if not isinstance(alloc, mybir.MemoryLocationSet):
                continue
            name = alloc.memorylocations[0].name
            if alloc.kind == "ExternalInput":
                if name != partition_name:
                    in_names.append(name)
            elif alloc.kind == "ExternalOutput":
                out_names.append(name)
                out_avals.append(jax.core.ShapedArray(
                    tuple(alloc.tensor_shape), mybir.dt.np(alloc.dtype)))
        self.in_params = list(in_names)
        self.out_names = out_names
        self.out_avals = out_avals
        all_names = list(in_names) + list(out_names)
        if partition_name is not None:
            all_names.append(partition_name)

        def _body(*args):
            operands = list(args)
            if partition_name is not None:
                operands.append(bass2jax.partition_id_tensor())
            outs = bass2jax._bass_exec_p.bind(
                *operands,
                out_avals=tuple(out_avals),
                in_names=tuple(all_names),
                out_names=tuple(out_names),
                lowering_input_output_aliases=(),
                sim_require_finite=True,
                sim_require_nnan=True,
                nc=nc,
            )
            return tuple(outs)

        devices = jax.devices()[:n_cores]
        self.mesh = Mesh(np.asarray(devices), ("core",))
        nin = len(self.in_params) + len(out_names)
        self.sharding = NamedSharding(self.mesh, PartitionSpec("core"))
        self.fn = jax.jit(shard_map(
            _body, mesh=self.mesh,
            in_specs=(PartitionSpec("core"),) * nin,
            out_specs=(PartitionSpec("core"),) * len(out_names),
            check_rep=False), keep_unused=True)
        self.zeros = None

    def put_inputs(self, in_maps):
        """Concatenate per-core inputs and commit to devices."""
        concat = [np.concatenate([np.asarray(m[k]) for m in in_maps], axis=0)
                  for k in self.in_params]
        args = [self.jax.device_put(a, self.sharding) for a in concat]
        if self.zeros is None:
            self.zeros = [
                self.jax.device_put(
                    np.zeros((self.n_cores * av.shape[0],) + av.shape[1:],
                             av.dtype), self.sharding)
                for av in self.out_avals]
        return args

    def run(self, args):
        return self.fn(*args, *self.zeros)

    def run_to_numpy(self, args):
        outs = self.run(args)
        res = []
        for c in range(self.n_cores):
            res.append({
                name: np.asarray(outs[i]).reshape(
                    (self.n_cores,) + self.out_avals[i].shape)[c]
                for i, name in enumerate(self.out_names)})
        return res


def kernel(**inputs):
    global LAST_RESULT
    cfg = Cfg()
    prep = _preprocess(cfg, inputs)
    key = prep["key"]
    ent = _CACHE.get(key)
    if ent is None:
        nc = _build(cfg, prep["meta"])
        ent = {"nc": nc, "runner": _Runner(nc, cfg.NCORE)}
        _CACHE[key] = ent
    runner = ent["runner"]
    args = runner.put_inputs(prep["in_maps"])
    results = runner.run_to_numpy(args)
    ent["last_args"] = args
    LAST_RESULT = ent
    interior = np.concatenate(
        [results[c]["out_int"][:cfg.NPC] for c in range(cfg.NCORE)], axis=0)
    boundary = np.concatenate(
        [results[c]["out_b"][:cfg.BPC] for c in range(cfg.NCORE)], axis=0)
    control = np.concatenate(
        [results[c]["out_c"][:cfg.CPC] for c in range(cfg.NCORE)], axis=0)
    return (np.ascontiguousarray(interior, dtype=np.float32),
            np.ascontiguousarray(boundary, dtype=np.float32),
            np.ascontiguousarray(control, dtype=np.float32))


def _tiny_runner(ncore=8):
    """Trivial 8-core NEFF (one small DMA in/out) to calibrate dispatch floor."""
    f32 = mybir.dt.float32
    nc = bacc.Bacc("TRN2", target_bir_lowering=False, num_devices=ncore)
    a = nc.declare_dram_parameter("a", [128, 256], f32, isOutput=False)
    o = nc.declare_dram_parameter("o", [128, 256], f32, isOutput=True)
    with tile.TileContext(nc) as tc, ExitStack() as ctx:
        pool = ctx.enter_context(tc.tile_pool(name="pool", bufs=1))
        t = pool.tile([128, 256], f32)
        nc.sync.dma_start(t[:], a[:])
        nc.sync.dma_start(o[:], t[:])
    nc.finalize()
    r = _Runner(nc, ncore)
    args = r.put_inputs([{"a": np.zeros((128, 256), np.float32)}] * ncore)
    return r, args


def _calib_runner(cfg, meta, in_maps):
    """Same parameter/output signature as the real kernel, trivial body —
    isolates per-call arg-marshal overhead from NEFF execution time."""
    N, D, DC, DH = cfg.N, cfg.D, cfg.DC, cfg.DH
    P, NPAD = cfg.P, cfg.NPAD
    f32, bf16, i16 = mybir.dt.float32, mybir.dt.bfloat16, mybir.dt.int16
    H = cfg.H
    nc = bacc.Bacc("TRN2", target_bir_lowering=False, num_devices=cfg.NCORE)
    dp = nc.declare_dram_parameter
    dp("x_lo", [H, D], f32, isOutput=False)
    dp("x_hi", [max(N - H, 1), D], f32, isOutput=False)
    dp("xb_tab", [cfg.EB, D], f32, isOutput=False)
    dp("u_tab", [cfg.EC, DC], f32, isOutput=False)
    x_sl = dp("x_sl", [NPAD, D], f32, isOutput=False)
    dp("xb_sl", [cfg.BPAD, D], f32, isOutput=False)
    dp("u_sl", [cfg.CPAD, DC], f32, isOutput=False)
    for k in ("rtot_b", "rint_b", "rb_b", "rc_b"):
        dp(k, [P, NPAD], bf16, isOutput=False)
    dp("scl4", [4, NPAD], bf16, isOutput=False)
    for k in STREAMS:
        NB = meta[k][1]
        dp(f"idx_{k}", [128, NB * 8], i16, isOutput=False)
        dp(f"tgtl_{k}", [P, NB], bf16, isOutput=False)
    wshapes = {
        "wisT": [D, DH], "f1T": [D, DH], "f2T": [D, DH], "f3T": [D, DH],
        "g1T": [D, DH], "g2T": [D, DH], "g3T": [DC, DH], "bias4": [4, DH],
        "wbT": [D, DH], "beff": [1, DH], "wcT": [DC, DH], "ceff": [1, DH],
    }
    for k, s in wshapes.items():
        dp(k, s, bf16, isOutput=False)
    out_i = dp("out_int", [NPAD, DH], f32, isOutput=True)
    dp("out_b", [cfg.BPAD, DH], f32, isOutput=True)
    dp("out_c", [cfg.CPAD, DH], f32, isOutput=True)
    with tile.TileContext(nc) as tc, ExitStack() as ctx:
        pool = ctx.enter_context(tc.tile_pool(name="pool", bufs=1))
        t = pool.tile([128, D], f32)
        nc.sync.dma_start(t[:], x_sl[0:128, :])
        nc.sync.dma_start(out_i[0:128, 0:D], t[:])
    nc.finalize()
    r = _Runner(nc, cfg.NCORE)
    args = r.put_inputs(in_maps)
    return r, args
